# revision 11
# baseline (speedup 1.0000x reference)
"""PinGNN Trainium2 kernel — 8-core SPMD Bass implementation.

Strategy (dst-sharded edges, no scatter in the hot path):
  - Per core: 6250-node shard; edges partitioned by dst.
  - Layer-1 neighbor sum  SB[n] = sum_{e: dst=n} B1[src[e]]  is done as a
    padded fixed-K dma_gather from an HBM B1 table (int16 indices; the
    50k-row table is addressed as two halves, each half's gather structure
    independently degree-sorted), followed by in-SBUF tree reductions.
    The second half's result is merged back through a small permute-gather.
  - Per-edge edge_attr work is pre-aggregated:  EA[n] = sum edge_attr[e],
    so each layer's edge-attr term is just EA @ W_eff[128:144].
  - The dst-gather of x (x[dst] term) collapses to deg[n] * (x@Wa + b_eff).
  - Layer 2 only needs graph-level sums -> everything becomes small matmuls
    against host-built one-hot/count matrices (gsel, cnt2), with a single
    [64,64] AllReduce at the end.
"""

import os

import numpy as np

# --- problem constants (hardcoded; kernel.py must be self-contained) ---
N = 50000
E = 800000
CE = 16
NG = 64
NC = 8
P = 128
SHARD = N // NC             # 6250
NCH = (SHARD + P - 1) // P  # 49
SLOTS = NCH * P             # 6272
HALF = 32767                # src < HALF via gather0; src >= HALF via gather1
GW = 32                     # column budget per gather bin
MAX_BIN_CH = 8              # chunks per bin cap (bounds comb staging)
F32 = np.float32

_CACHE = {}
_DEBUG_DUMPS = False


def _build_debug(plan):
    global _DEBUG_DUMPS
    _DEBUG_DUMPS = True
    try:
        return _build(plan)
    finally:
        _DEBUG_DUMPS = False


# ----------------------------------------------------------------------
# host-side preprocessing
# ----------------------------------------------------------------------

def _pack_idx16(idx32):
    """[128, C] int32 -> wrapped int16 [128, C*8] for dma_gather.

    dma_gather flat order j = w*128 + p (dst slot [j%128, j//128]); the
    wrapped tensor holds flat j at [j%16, j//16], replicated to 128 parts.
    """
    Pp, C = idx32.shape
    assert Pp == P
    w16 = np.transpose(idx32.reshape(8, 16, C), (1, 2, 0)).reshape(16, C * 8)
    return np.tile(w16.astype(np.int16), (8, 1))


def _pack_flat_idx16(flat):
    return np.tile(flat.astype(np.int16).reshape(-1, 16).T, (8, 1))


def _make_bins(Ks):
    """Group chunks (ascending K) into gather bins of <= GW columns."""
    bins = []  # (c0, nch, kq)
    c = 0
    n = len(Ks)
    while c < n:
        kq = max(Ks[c], 1)
        nch = 1
        while (
            c + nch < n
            and nch < MAX_BIN_CH
            and (nch + 1) * max(kq, Ks[c + nch], 1) <= GW
        ):
            kq = max(kq, Ks[c + nch], 1)
            nch += 1
        assert kq <= 2 * GW, f"chunk K={kq} too large"
        bins.append((c, nch, kq))
        c += nch
    return bins


def _bin_offsets(bins):
    off = np.zeros(NCH, np.int64)
    o = 0
    for c0, nch, kq in bins:
        for i in range(nch):
            off[c0 + i] = o + i * kq
        o += nch * kq
    return off, int(o)


def _host_prep(inputs):
    x = np.ascontiguousarray(np.asarray(inputs["x"], F32))
    ea_full = np.ascontiguousarray(np.asarray(inputs["edge_attr"], F32))
    ei = np.asarray(inputs["edge_index"]).astype(np.int64)
    gi = np.asarray(inputs["graph_index"]).astype(np.int64)
    src, dst = ei[0], ei[1]

    cores_raw = []
    K0_all = np.zeros((NC, NCH), np.int64)
    K1_all = np.zeros((NC, NCH), np.int64)
    for k in range(NC):
        n0 = k * SHARD
        emask = (dst >= n0) & (dst < n0 + SHARD)
        esrc = src[emask]
        edst = (dst[emask] - n0).astype(np.int64)
        eidx = np.nonzero(emask)[0]

        deg = np.bincount(edst, minlength=SHARD)
        half1 = esrc >= HALF
        deg0 = np.bincount(edst[~half1], minlength=SHARD)
        deg1 = np.bincount(edst[half1], minlength=SHARD)
        pi0 = np.argsort(deg0, kind="stable")
        pi1 = np.argsort(deg1, kind="stable")

        d0s = deg0[pi0]
        d1s = deg1[pi1]
        for c in range(NCH):
            lo, hi = c * P, min((c + 1) * P, SHARD)
            if lo < SHARD:
                K0_all[k, c] = d0s[lo:hi].max()
                K1_all[k, c] = d1s[lo:hi].max()
        cores_raw.append(
            dict(n0=n0, esrc=esrc, edst=edst, eidx=eidx, deg=deg,
                 half1=half1, pi0=pi0, pi1=pi1))

    K0c = np.maximum(K0_all.max(0), 1)
    K1c = np.maximum(K1_all.max(0), 1)
    bins0 = _make_bins(list(K0c))
    bins1 = _make_bins(list(K1c))
    off0, C0 = _bin_offsets(bins0)
    off1, C1 = _bin_offsets(bins1)
    plan = dict(bins0=bins0, bins1=bins1, C0=C0, C1=C1)

    pad0 = 0                # table row 0 is a zero row (half-0 frame)
    pad1 = N + 1 - HALF     # table row N+1 is a zero row (half-1 frame)

    xT = np.ascontiguousarray(x.T)  # [64, N], shared

    gdst = gi[dst]
    cnt_mat = np.zeros((N, NG), F32)
    np.add.at(cnt_mat, (src, gdst), 1.0)

    in_maps = []
    for k, co in enumerate(cores_raw):
        pi0, pi1 = co["pi0"], co["pi1"]
        inv0 = np.empty(SHARD, np.int64)
        inv0[pi0] = np.arange(SHARD)
        inv1 = np.empty(SHARD, np.int64)
        inv1[pi1] = np.arange(SHARD)

        def structure(sel, inv, off, C, idx_map, padv):
            s_src = co["esrc"][sel]
            s_dst = co["edst"][sel]
            s_ei = co["eidx"][sel]
            order = np.argsort(s_dst, kind="stable")
            s_src, s_dst, s_ei = s_src[order], s_dst[order], s_ei[order]
            counts = np.bincount(s_dst, minlength=SHARD)
            starts = np.zeros(SHARD, np.int64)
            np.cumsum(counts[:-1], out=starts[1:])
            rank = np.arange(len(s_dst)) - starts[s_dst]
            slot = inv[s_dst]
            lane = slot % P
            col = off[slot // P] + rank
            idx = np.full((P, C), padv, np.int32)
            idx[lane, col] = idx_map(s_src)
            ealay = np.zeros((P, C, CE), F32)
            ealay[lane, col] = ea_full[s_ei]
            return idx, ealay

        idx0, ea0 = structure(~co["half1"], inv0, off0, C0,
                              lambda s: (s + 1).astype(np.int32), pad0)
        idx1, ea1 = structure(co["half1"], inv1, off1, C1,
                              lambda s: (s + 1 - HALF).astype(np.int32), pad1)

        # permute-gather: canonical slot s -> r1buf row (p1 * NCH + c1)
        permi = np.zeros(SLOTS, np.int64)
        s1 = inv1[pi0]
        permi[:SHARD] = (s1 % P) * NCH + (s1 // P)
        idxp = _pack_flat_idx16(permi)

        def slotify(a):
            out = np.zeros((SLOTS,) + a.shape[1:], a.dtype)
            out[:SHARD] = a[pi0]
            return out

        n0 = co["n0"]
        xpT = np.ascontiguousarray(slotify(x[n0 : n0 + SHARD]).T)
        deg_l = np.ascontiguousarray(
            slotify(co["deg"].astype(F32)).reshape(NCH, P).T)

        gsel = np.zeros((SLOTS, NG), F32)
        gsel[np.arange(SHARD), gi[n0 : n0 + SHARD][pi0]] = 1.0
        gsel_l = np.ascontiguousarray(gsel.reshape(NCH, P, NG).transpose(1, 0, 2))
        cnt2 = np.zeros((SLOTS, NG), F32)
        cnt2[:SHARD] = cnt_mat[pi0 + n0]
        cnt2_l = np.ascontiguousarray(cnt2.reshape(NCH, P, NG).transpose(1, 0, 2))

        in_maps.append(dict(
            xT=xT, xpT=xpT, deg=deg_l, gsel=gsel_l, cnt2=cnt2_l,
            idx0=_pack_idx16(idx0), idx1=_pack_idx16(idx1), idxp=idxp,
            ea0=ea0, ea1=ea1,
        ))

    W1e = np.asarray(inputs["W1_eff"], F32)
    W2e = np.asarray(inputs["W2_eff"], F32)
    W1o = np.asarray(inputs["W1_out"], F32)
    W2o = np.asarray(inputs["W2_out"], F32)
    cnt_g = np.bincount(gi, minlength=NG).astype(F32)
    shared = dict(
        W1a=np.ascontiguousarray(W1e[:64]),
        W1b=np.ascontiguousarray(W1e[64:128]),
        W1c=np.ascontiguousarray(W1e[128:]),
        W2a=np.ascontiguousarray(W2e[:64]),
        W2b=np.ascontiguousarray(W2e[64:128]),
        W2c=np.ascontiguousarray(W2e[128:]),
        W1oa=np.ascontiguousarray(W1o[:64]),
        W1ob=np.ascontiguousarray(W1o[64:]),
        W2oa=np.ascontiguousarray(W2o[:64]),
        W2ob=np.ascontiguousarray(W2o[64:]),
        b1e_b=np.tile(np.asarray(inputs["b1_eff"], F32)[None, :], (P, 1)),
        b2e_b=np.tile(np.asarray(inputs["b2_eff"], F32)[None, :], (P, 1)),
        b1o_c=np.ascontiguousarray(np.asarray(inputs["b1_out"], F32)[:, None]),
        b2o_b=np.tile(np.asarray(inputs["b2_out"], F32)[None, :], (NG, 1)),
        cnt_c=np.ascontiguousarray(np.maximum(cnt_g, 1.0)[:, None]),
        cnt_raw=np.ascontiguousarray(cnt_g[:, None]),
    )
    for m in in_maps:
        m.update(shared)
    return plan, in_maps


# ----------------------------------------------------------------------
# bass program
# ----------------------------------------------------------------------

def _build(plan):
    import contextlib

    import concourse.bacc as bacc
    import concourse.mybir as mybir
    import concourse.tile as tile
    from concourse.masks import make_identity

    f32 = mybir.dt.float32
    i16 = mybir.dt.int16
    AF = mybir.ActivationFunctionType
    OP = mybir.AluOpType
    C0, C1 = plan["C0"], plan["C1"]
    bins0, bins1 = plan["bins0"], plan["bins1"]
    max_cols = max(
        [kq * nch for (_, nch, kq) in bins0 + bins1])
    max_nch = max([nch for (_, nch, kq) in bins0 + bins1])

    nc = bacc.Bacc(None, target_bir_lowering=False, debug=False,
                   num_swdge_queues=4)

    din = {}
    for name, shape in [
        ("xT", [64, N]), ("xpT", [64, SLOTS]), ("deg", [P, NCH]),
        ("gsel", [P, NCH, NG]), ("cnt2", [P, NCH, NG]),
        ("ea0", [P, C0, CE]), ("ea1", [P, C1, CE]),
        ("W1a", [64, 64]), ("W1b", [64, 64]), ("W1c", [CE, 64]),
        ("W2a", [64, 64]), ("W2b", [64, 64]), ("W2c", [CE, 64]),
        ("W1oa", [64, 64]), ("W1ob", [64, 64]),
        ("W2oa", [64, 64]), ("W2ob", [64, 64]),
        ("b1e_b", [P, 64]), ("b2e_b", [P, 64]), ("b1o_c", [64, 1]),
        ("b2o_b", [NG, 64]), ("cnt_c", [NG, 1]), ("cnt_raw", [NG, 1]),
    ]:
        din[name] = nc.dram_tensor(name, shape, f32, kind="ExternalInput")
    for name, cols in [("idx0", C0 * 8), ("idx1", C1 * 8),
                       ("idxp", SLOTS // 16)]:
        din[name] = nc.dram_tensor(name, [P, cols], i16, kind="ExternalInput")

    out_t = nc.dram_tensor("out", [NG, 64], f32, kind="ExternalOutput")
    dbg = {}
    if _DEBUG_DUMPS:
        dbg["SB1"] = nc.dram_tensor(
            "dbg_SB1", [P, NCH * 64], f32, kind="ExternalOutput")
        dbg["EA"] = nc.dram_tensor(
            "dbg_EA", [P, NCH * CE], f32, kind="ExternalOutput")
        dbg["h1T"] = nc.dram_tensor(
            "dbg_h1T", [64, SLOTS], f32, kind="ExternalOutput")
        dbg["EAT"] = nc.dram_tensor(
            "dbg_EAT", [CE, SLOTS], f32, kind="ExternalOutput")
        dbg["agg"] = nc.dram_tensor(
            "dbg_agg", [P, NCH * 64], f32, kind="ExternalOutput")
    ar_in = nc.dram_tensor("ar_in", [NG, 64], f32)
    ar_out = nc.dram_tensor("ar_out", [NG, 64], f32, addr_space="Shared")
    b1tab = nc.dram_tensor("b1tab", [N + 2, 64], f32)
    r1buf = nc.dram_tensor("r1buf", [SLOTS, P], f32)

    with tile.TileContext(nc) as tc, contextlib.ExitStack() as ctx:
        res = ctx.enter_context(tc.tile_pool(name="res", bufs=1))
        xtp = ctx.enter_context(tc.tile_pool(name="xtp", bufs=3))
        b1p = ctx.enter_context(tc.tile_pool(name="b1p", bufs=4))
        gp = ctx.enter_context(tc.tile_pool(name="gp", bufs=4))
        eap = ctx.enter_context(tc.tile_pool(name="eap", bufs=3))
        ixp = ctx.enter_context(tc.tile_pool(name="ixp", bufs=3))
        combp = ctx.enter_context(tc.tile_pool(name="combp", bufs=3))
        stg = ctx.enter_context(tc.tile_pool(name="stg", bufs=4))
        gcp = ctx.enter_context(tc.tile_pool(name="gcp", bufs=3))
        psum = ctx.enter_context(tc.tile_pool(name="psum", bufs=6, space="PSUM"))
        psF_p = ctx.enter_context(tc.tile_pool(name="psFp", bufs=1, space="PSUM"))

        # ---- resident small tensors ----
        wsb = {}
        for nm in ["W1a", "W1b", "W1c", "W2a", "W2b", "W2c",
                   "W1oa", "W1ob", "W2oa", "W2ob",
                   "b1e_b", "b2e_b", "b1o_c", "b2o_b", "cnt_c", "cnt_raw"]:
            t = res.tile(list(din[nm].shape), f32, name=f"sb_{nm}")
            nc.sync.dma_start(out=t[:], in_=din[nm][:])
            wsb[nm] = t

        ident = res.tile([P, P], f32, name="ident")
        make_identity(nc, ident[:])

        deg_sb = res.tile([P, NCH], f32, name="deg_sb")
        nc.sync.dma_start(out=deg_sb[:], in_=din["deg"][:])
        idxp_sb = res.tile([P, SLOTS // 16], i16, name="idxp_sb")
        nc.sync.dma_start(out=idxp_sb[:], in_=din["idxp"][:])

        SB1 = res.tile([P, NCH * 64], f32, name="SB1")
        EAs = res.tile([P, NCH * CE], f32, name="EAs")
        EA_T = res.tile([CE, SLOTS], f32, name="EA_T")
        h1T = res.tile([64, SLOTS], f32, name="h1T")
        sb1v = SB1[:].rearrange("p (c f) -> p c f", f=64)
        eav = EAs[:].rearrange("p (c f) -> p c f", f=CE)

        zrow = res.tile([1, 64], f32, name="zrow")
        nc.gpsimd.memset(zrow[:], 0.0)

        # ---- phase B: B1 table = [0 ; x @ W1b ; 0] ----
        nc.sync.dma_start(out=b1tab[0:1, :], in_=zrow[:])
        nc.sync.dma_start(out=b1tab[N + 1 : N + 2, :], in_=zrow[:])
        for t in range((N + 511) // 512):
            lo = t * 512
            hi = min(lo + 512, N)
            xt = xtp.tile([64, 512], f32, tag="xt")
            nc.sync.dma_start(out=xt[:, : hi - lo], in_=din["xT"][:, lo:hi])
            for j in range((hi - lo + P - 1) // P):
                b0 = lo + j * P
                rows = min(P, N - b0)
                ps = psum.tile([P, 64], f32, tag="ps")
                nc.tensor.matmul(
                    out=ps[:rows, :], lhsT=xt[:, j * P : j * P + rows],
                    rhs=wsb["W1b"][:], start=True, stop=True)
                sb = b1p.tile([P, 64], f32, tag="b1sb")
                nc.vector.tensor_copy(out=sb[:rows, :], in_=ps[:rows, :])
                nc.sync.dma_start(
                    out=b1tab[1 + b0 : 1 + b0 + rows, :], in_=sb[:rows, :])

        # ---- gathers + tree reductions ----
        qn = [0]

        def gather_half(bins, idx_dram, tab_ap, ea_in, to_r1):
            o = 0
            for (c0, nch, kq) in bins:
                cols = nch * kq
                ix = ixp.tile([P, max_cols * 8], i16, tag="ix")
                nc.sync.dma_start(
                    out=ix[:, : cols * 8],
                    in_=idx_dram[:, o * 8 : (o + cols) * 8])
                g = gp.tile([P, max_cols * 64], f32, tag="G")
                g3 = g[:, : cols * 64].rearrange("p (c f) -> p c f", f=64)
                g4 = g[:, : cols * 64].rearrange(
                    "p (c k f) -> p c k f", k=kq, f=64)
                nidx = P * cols
                nc.gpsimd.dma_gather(
                    g3, tab_ap, ix[:, : cols * 8], nidx, nidx, 64,
                    single_packet=False, queue_num=qn[0] % 4)
                qn[0] += 1
                et = eap.tile([P, max_cols * CE], f32, tag="EAt")
                nc.sync.dma_start(
                    out=et[:, : cols * CE],
                    in_=ea_in[:, o : o + cols, :].rearrange("p c f -> p (c f)"))
                ev = et[:, : cols * CE].rearrange(
                    "p (c k f) -> p c k f", k=kq, f=CE)
                K = kq
                while K > 1:
                    h = K // 2
                    nc.vector.tensor_tensor(
                        out=g4[:, :, 0:h, :], in0=g4[:, :, 0:h, :],
                        in1=g4[:, :, K - h : K, :], op=OP.add)
                    nc.vector.tensor_tensor(
                        out=ev[:, :, 0:h, :], in0=ev[:, :, 0:h, :],
                        in1=ev[:, :, K - h : K, :], op=OP.add)
                    K -= h
                if not to_r1:
                    nc.vector.tensor_copy(
                        out=sb1v[:, c0 : c0 + nch, :], in_=g4[:, :, 0, :])
                    nc.vector.tensor_copy(
                        out=eav[:, c0 : c0 + nch, :], in_=ev[:, :, 0, :])
                else:
                    cb = combp.tile([P, max_nch * P], f32, tag="comb")
                    cbv = cb[:, : nch * P].rearrange("p (c f) -> p c f", f=P)
                    nc.vector.tensor_copy(
                        out=cbv[:, :, 0:64], in_=g4[:, :, 0, :])
                    nc.vector.tensor_copy(
                        out=cbv[:, :, 64 : 64 + CE], in_=ev[:, :, 0, :])
                    nc.sync.dma_start(
                        out=r1buf[:].rearrange(
                            "(p c) f -> p c f", c=NCH)[:, c0 : c0 + nch, :],
                        in_=cbv)
                o += cols

        gather_half(bins0, din["idx0"][:], b1tab[0 : HALF + 1, :],
                    din["ea0"][:], to_r1=False)
        gather_half(bins1, din["idx1"][:], b1tab[HALF : N + 2, :],
                    din["ea1"][:], to_r1=True)

        # permute-merge of half-1 results (two halves of chunks)
        rpp = ctx.enter_context(tc.tile_pool(name="rpp", bufs=2))
        for (c0, c1) in [(0, 25), (25, NCH)]:
            nchh = c1 - c0
            rp = rpp.tile([P, 25 * P], f32, tag="rp")
            rpv = rp[:, : nchh * P].rearrange("p (c f) -> p c f", f=P)
            nidx = nchh * P
            nc.gpsimd.dma_gather(
                rpv, r1buf[:], idxp_sb[:, c0 * 8 : c1 * 8], nidx, nidx, P,
                single_packet=False, queue_num=qn[0] % 4)
            qn[0] += 1
            nc.vector.tensor_tensor(
                out=sb1v[:, c0:c1, :], in0=sb1v[:, c0:c1, :],
                in1=rpv[:, :, 0:64], op=OP.add)
            nc.vector.tensor_tensor(
                out=eav[:, c0:c1, :], in0=eav[:, c0:c1, :],
                in1=rpv[:, :, 64 : 64 + CE], op=OP.add)

        if _DEBUG_DUMPS:
            nc.sync.dma_start(out=dbg["SB1"][:], in_=SB1[:])
            nc.sync.dma_start(out=dbg["EA"][:], in_=EAs[:])

        # ---- EA transpose (feat-major) ----
        for c in range(NCH):
            pt = psum.tile([CE, P], f32, tag="ps")
            nc.tensor.transpose(out=pt[:], in_=eav[:, c, :], identity=ident[:])
            nc.vector.tensor_copy(out=EA_T[:, c * P : (c + 1) * P], in_=pt[:])

        # ---- layer 1 (per chunk) ----
        for c in range(NCH):
            xpc = xtp.tile([64, P], f32, tag="xpc")
            nc.sync.dma_start(out=xpc[:], in_=din["xpT"][:, c * P : (c + 1) * P])
            ps = psum.tile([P, 64], f32, tag="ps")
            nc.tensor.matmul(out=ps[:], lhsT=xpc[:], rhs=wsb["W1a"][:],
                             start=True, stop=True)
            psE1 = psum.tile([P, 64], f32, tag="ps")
            nc.tensor.matmul(out=psE1[:], lhsT=EA_T[:, c * P : (c + 1) * P],
                             rhs=wsb["W1c"][:], start=True, stop=True)
            ag = stg.tile([P, 64], f32, tag="agg")
            nc.vector.tensor_tensor(
                out=ag[:], in0=ps[:], in1=wsb["b1e_b"][:], op=OP.add)
            nc.vector.tensor_scalar_mul(
                out=ag[:], in0=ag[:], scalar1=deg_sb[:, c : c + 1])
            nc.vector.tensor_tensor(
                out=ag[:], in0=ag[:], in1=psE1[:], op=OP.add)
            nc.vector.tensor_tensor(
                out=ag[:], in0=ag[:], in1=sb1v[:, c, :], op=OP.add)
            if _DEBUG_DUMPS:
                nc.sync.dma_start(
                    out=dbg["agg"][:, c * 64 : (c + 1) * 64], in_=ag[:])
            pt = psum.tile([64, P], f32, tag="ps")
            nc.tensor.transpose(out=pt[:], in_=ag[:], identity=ident[:])
            agT = stg.tile([64, P], f32, tag="agT")
            nc.vector.tensor_copy(out=agT[:], in_=pt[:])
            ph = psum.tile([64, P], f32, tag="ps")
            nc.tensor.matmul(out=ph[:], lhsT=wsb["W1oa"][:], rhs=xpc[:],
                             start=True, stop=False)
            nc.tensor.matmul(out=ph[:], lhsT=wsb["W1ob"][:], rhs=agT[:],
                             start=False, stop=True)
            nc.scalar.activation(
                out=h1T[:, c * P : (c + 1) * P], in_=ph[:],
                func=AF.Relu, bias=wsb["b1o_c"][:])

        if _DEBUG_DUMPS:
            nc.sync.dma_start(out=dbg["h1T"][:], in_=h1T[:])
            nc.sync.dma_start(out=dbg["EAT"][:], in_=EA_T[:])

        # ---- layer 2 (per chunk) + graph accumulation ----
        psF = psF_p.tile([NG, 64], f32, name="psF")
        psF2 = psF_p.tile([64, NG], f32, name="psF2")
        for c in range(NCH):
            h1c = h1T[:, c * P : (c + 1) * P]
            psA = psum.tile([P, 64], f32, tag="ps")
            nc.tensor.matmul(out=psA[:], lhsT=h1c, rhs=wsb["W2a"][:],
                             start=True, stop=True)
            psE = psum.tile([P, 64], f32, tag="ps")
            nc.tensor.matmul(out=psE[:], lhsT=EA_T[:, c * P : (c + 1) * P],
                             rhs=wsb["W2c"][:], start=True, stop=True)
            z = stg.tile([P, 64], f32, tag="z")
            nc.vector.tensor_tensor(
                out=z[:], in0=psA[:], in1=wsb["b2e_b"][:], op=OP.add)
            nc.vector.tensor_scalar_mul(
                out=z[:], in0=z[:], scalar1=deg_sb[:, c : c + 1])
            nc.vector.tensor_tensor(out=z[:], in0=z[:], in1=psE[:], op=OP.add)
            pt = psum.tile([64, P], f32, tag="ps")
            nc.tensor.transpose(out=pt[:], in_=z[:], identity=ident[:])
            zT = stg.tile([64, P], f32, tag="zT")
            nc.vector.tensor_copy(out=zT[:], in_=pt[:])
            psU = psum.tile([P, 64], f32, tag="ps")
            nc.tensor.matmul(out=psU[:], lhsT=h1c, rhs=wsb["W2oa"][:],
                             start=True, stop=False)
            nc.tensor.matmul(out=psU[:], lhsT=zT[:], rhs=wsb["W2ob"][:],
                             start=False, stop=True)
            uc = stg.tile([P, 64], f32, tag="uc")
            nc.vector.tensor_copy(out=uc[:], in_=psU[:])
            psB = psum.tile([P, 64], f32, tag="ps")
            nc.tensor.matmul(out=psB[:], lhsT=h1c, rhs=wsb["W2b"][:],
                             start=True, stop=True)
            bc = stg.tile([P, 64], f32, tag="bc")
            nc.vector.tensor_copy(out=bc[:], in_=psB[:])

            gselc = gcp.tile([P, NG], f32, tag="gselc")
            nc.sync.dma_start(out=gselc[:], in_=din["gsel"][:, c, :])
            cntc = gcp.tile([P, NG], f32, tag="cntc")
            nc.sync.dma_start(out=cntc[:], in_=din["cnt2"][:, c, :])
            nc.tensor.matmul(out=psF[:], lhsT=gselc[:], rhs=uc[:],
                             start=(c == 0), stop=False,
                             skip_group_check=True)
            nc.tensor.matmul(out=psF2[:], lhsT=bc[:], rhs=cntc[:],
                             start=(c == 0), stop=(c == NCH - 1),
                             skip_group_check=True)

        sbS = stg.tile([64, NG], f32, tag="sbS")
        nc.vector.tensor_copy(out=sbS[:], in_=psF2[:])
        nc.tensor.matmul(out=psF[:], lhsT=sbS[:], rhs=wsb["W2ob"][:],
                         start=False, stop=True, skip_group_check=True)
        sbF = stg.tile([NG, 64], f32, tag="sbF")
        nc.vector.tensor_copy(out=sbF[:], in_=psF[:])
        nc.sync.dma_start(out=ar_in[:], in_=sbF[:])
        nc.gpsimd.collective_compute(
            "AllReduce", OP.add, replica_groups=[list(range(NC))],
            ins=[ar_in[:]], outs=[ar_out[:]])
        sbAR = stg.tile([NG, 64], f32, tag="sbAR")
        nc.sync.dma_start(out=sbAR[:], in_=ar_out[:])
        tb = stg.tile([NG, 64], f32, tag="tb")
        nc.vector.tensor_scalar_mul(
            out=tb[:], in0=wsb["b2o_b"][:], scalar1=wsb["cnt_raw"][:])
        nc.vector.tensor_tensor(out=sbAR[:], in0=sbAR[:], in1=tb[:], op=OP.add)
        inv = stg.tile([NG, 1], f32, tag="inv")
        nc.vector.reciprocal(out=inv[:], in_=wsb["cnt_c"][:])
        nc.vector.tensor_scalar_mul(out=sbAR[:], in0=sbAR[:], scalar1=inv[:])
        nc.sync.dma_start(out=out_t[:], in_=sbAR[:])

    nc.finalize()
    return nc


# ----------------------------------------------------------------------
# entry point
# ----------------------------------------------------------------------

def kernel(**inputs) -> np.ndarray:
    plan, in_maps = _host_prep(inputs)
    key = (tuple(plan["bins0"]), tuple(plan["bins1"]), plan["C0"], plan["C1"])
    if _CACHE.get("key") != key:
        _CACHE["nc"] = _build(plan)
        _CACHE["key"] = key
    nc = _CACHE["nc"]

    from concourse.bass_utils import run_bass_kernel_spmd

    trace = bool(os.environ.get("PINGNN_TRACE"))
    res = run_bass_kernel_spmd(nc, in_maps, list(range(NC)), trace=trace)
    _CACHE["last_result"] = res
    return np.asarray(res.results[0]["out"], F32)
